# revision 1
# baseline (speedup 1.0000x reference)
"""Trainium2 Bass kernel for nn_CaserQueryEncoder.

Model (B=1024, L=50, D=128, NV=8, NH=16):
  P_u = user_emb[user_ids]                                   [B, D]
  E   = item_emb[item_seq]                                   [B, L, D]
  o_v = einsum('btd,vt->bvd', E, Wv) + bv                    [B, NV*D]
  conv[b,i,j,t] = sum_{dt<=i} <E[b, t+dt, :], Wh[i,j,dt,:]>  (Wh zero for dt>i)
  o_h[b,i,j] = max over valid t (t <= 49-i) of relu(conv + bh)
  z = relu([o_v, o_h] @ fc_W + fc_b)                         [B, D]
  out = [z, P_u]                                             [B, 2D]

Strategy: pure data parallel, 128 batch rows per core x 8 cores, no
collectives. Per core:
  - indirect-DMA gather of E (6400 rows) and P_u; each gathered [b, d]
    block is cast to bf16 on the scalar engine, PE-transposed (bf16,
    1 cyc/row) and copied into ET[d, b, t'] (t' padded to 64 with zeros
    = the conv zero padding).
  - ALL matmul operands are bf16 (PSUM accumulation stays fp32): bf16
    enables the PE fast-weight-load path (f32r disables it) and halves
    the 14MB conv weight stream to 7MB.
  - horizontal conv as PSUM-accumulated shifted matmuls: heights packed
    8 per chunk (x16 filters = M=128 weight columns); for each tap dt the
    rhs is ET shifted by dt in t'; PSUM accumulates over dt. Invalid
    (height, t) positions get an additive -1e30 mask before the max-
    reduce. max(relu(x+b)) == relu(max(x)+b), so relu+bias happen after
    the max on the scalar engine.
  - conv weights stream as 28 tap-group DMAs (<=8 taps each) issued in
    the order the gather chase needs them, so the first chunks' weights
    land ~2µs after kernel start instead of ~30µs.
  - vertical conv never materialized: since o_v enters the fc linearly,
    G[t,d,k] = sum_v Wv[v,t]*fc_W[v*128+d,k] is precomputed on host and
    E @ G is added straight into the fc accumulation PSUM.
  - fc bias added via a K=1 ones-matmul.
"""

import math
import os
import sys
from contextlib import ExitStack

import numpy as np

sys.path.insert(0, "/opt/trn_rl_repo")

import ml_dtypes

import concourse.bass as bass
import concourse.tile as tile
from concourse import mybir
from concourse.bass import IndirectOffsetOnAxis
from concourse.bass_utils import run_bass_kernel_spmd
from concourse.masks import make_identity
from concourse.vector_clock import ScopedClock


def _patch_tile_drain():
    """This container's walrus codegen only accepts one sync-wait per Drain
    (CTRL_NO_STRUCT); Tile's kernel-tail drain carries one wait per live
    semaphore. Split the waits across a chain of drains, one wait each."""
    if getattr(tile.TileContext, "_drain_split_patched", False):
        return

    def _patched(self, tick_clock, wait_clock):
        nc = self.nc
        probe = nc.sync.drain()
        wait_clock.add_sem_waits(
            probe.ins, ScopedClock({None: tick_clock.global_clock}))
        nc.all_engine_barrier()
        popped = nc._tile_sem_poison_stack.pop()
        assert popped is self._sem_poison
        nc.clear_and_free_semaphores(list(self.sems.allocated().values()))
        nc.all_engine_barrier()

    tile.TileContext._drain_and_barrier = _patched
    tile.TileContext._drain_split_patched = True


_patch_tile_drain()


def _split_json_waits(j, max_waits=1):
    """This walrus codegen accepts at most one sync-wait per instruction.
    Hoist extra waits onto wait-only EventSemaphore instructions inserted
    just before the offender on the same engine queue."""
    n = 0
    for fn in j["functions"]:
        for blk in fn["blocks"]:
            out = []
            for inst in blk["instructions"]:
                si = inst.get("sync_info")
                waits = (si or {}).get("on_wait") or []
                if len(waits) > max_waits:
                    for k, w in enumerate(waits[:-max_waits]):
                        out.append({
                            "debug": inst.get("debug", 0),
                            "engine": inst["engine"],
                            "ins": [], "outs": [],
                            "name": f"{inst['name']}_wsplit{k}",
                            "opcode": "EventSemaphore",
                            "sync_info": {"on_update": [], "on_wait": [w]},
                        })
                        n += 1
                    si["on_wait"] = waits[-max_waits:]
                out.append(inst)
            blk["instructions"] = out
    return n


def _install_wait_splitter(nc):
    import json as _json

    orig = nc.to_json_bytes

    def patched():
        j = _json.loads(orig())
        _split_json_waits(j)
        return _json.dumps(j).encode()

    nc.to_json_bytes = patched

B = 1024
L = 50
D = 128
NV = 8
NH = 16
NU = 100000
NI = 500000
NCORES = 8
BLOC = B // NCORES          # 128 batch rows per core
TP = 64                     # t' pitch in ET (>= max dt + max Nt = 56)
NEG = -1.0e30
FC_IN = NV * D + NH * L     # 1824
NOUT = 2 * D                # 256
WGRP = 8                    # taps per weight-DMA group

# Height-chunk table: heights [8u, 8u+nh) packed as m2 = 16*(i-8u)+j.
# ndt taps accumulate in PSUM; Nt is the t-window (valid-t of the chunk's
# shortest filter); Nb batch rows per matmul so that Nb*Nt <= 512.
CHUNKS = []
_base = 0
for _u in range(7):
    _i0 = 8 * _u
    _nh = min(8, L - _i0)
    _ndt = min(_i0 + 8, L)
    _nt = L - _i0
    _nb = min(BLOC, 512 // _nt)
    _nblk = math.ceil(BLOC / _nb)
    CHUNKS.append(dict(i0=_i0, nh=_nh, ndt=_ndt, nt=_nt, nb=_nb,
                       nblk=_nblk, base=_base))
    _base += _ndt
NWTILES = _base             # 218 weight tiles of [d=128, m2=128]

# Weight-DMA groups: chunk u's taps split into ceil(ndt/WGRP) slices;
# group (u, k) covers taps [k*WGRP, min((k+1)*WGRP, ndt)). It is first
# needed when tap k*WGRP becomes runnable, i.e. at gather column
# min(k*WGRP + nt - 1, L-1). Stream the DMAs in need order.
WGROUPS = []                # (need_col, u, k, dt0, dt1)
for _u, _ch in enumerate(CHUNKS):
    for _k in range(math.ceil(_ch["ndt"] / WGRP)):
        _dt0 = _k * WGRP
        _dt1 = min(_dt0 + WGRP, _ch["ndt"])
        _need = min(_dt0 + _ch["nt"] - 1, L - 1)
        WGROUPS.append((_need, _u, _k, _dt0, _dt1))
WGROUPS.sort(key=lambda g: (g[0], -g[1]))
# trigger each group's DMA a few gather slots before it is needed
WTRIG = {}
for _g in WGROUPS:
    WTRIG.setdefault(max(0, _g[0] - 6), []).append(_g)

_NC_CACHE = None

# Set BASS_KERNEL_TRACE=1 to profile; exec time lands in LAST_RESULTS.
LAST_RESULTS = None


def _build_nc():
    f32 = mybir.dt.float32
    bf16 = mybir.dt.bfloat16
    i32 = mybir.dt.int32
    X = mybir.AxisListType.X
    Copy = mybir.ActivationFunctionType.Copy

    nc = bass.Bass()
    seq_t = nc.dram_tensor("seq_idx", [BLOC, L], i32, kind="ExternalInput")
    uid_t = nc.dram_tensor("uid_idx", [BLOC, 1], i32, kind="ExternalInput")
    item_t = nc.dram_tensor("item_emb", [NI, D], f32, kind="ExternalInput")
    user_t = nc.dram_tensor("user_emb", [NU, D], f32, kind="ExternalInput")
    whp_t = nc.dram_tensor("whp", [D, NWTILES * 128], bf16, kind="ExternalInput")
    g_t = nc.dram_tensor("g", [D, L * D], bf16, kind="ExternalInput")
    fcwh_t = nc.dram_tensor("fcwh", [128, 7 * D], bf16, kind="ExternalInput")
    masks_t = nc.dram_tensor("masks", [128, 7 * 512], f32, kind="ExternalInput")
    bh_t = nc.dram_tensor("bh_p", [128, 7], f32, kind="ExternalInput")
    fcb_t = nc.dram_tensor("fcb", [1, D], bf16, kind="ExternalInput")
    out_t = nc.dram_tensor("out", [BLOC, NOUT], f32, kind="ExternalOutput")

    # conv matmul (u, blk, dt) becomes runnable once ET column
    # min(dt + Nt - 1, L-1) is gathered (t' >= L is the zero pad).
    # These six groups chase the gather stream; the rest run after it.
    PHASE_A = [(6, 0), (5, 0), (5, 1), (5, 2), (4, 0), (4, 1)]

    with ExitStack() as ctx:
        tc = ctx.enter_context(tile.TileContext(nc))
        const = ctx.enter_context(tc.tile_pool(name="const", bufs=1))
        egath = ctx.enter_context(tc.tile_pool(name="egath", bufs=16))
        ebfp = ctx.enter_context(tc.tile_pool(name="ebfp", bufs=4))
        gpool = ctx.enter_context(tc.tile_pool(name="gpool", bufs=8))
        etp = ctx.enter_context(tc.tile_pool(name="etp", bufs=1))
        wpool = ctx.enter_context(tc.tile_pool(name="wpool", bufs=1))
        ohp = ctx.enter_context(tc.tile_pool(name="ohp", bufs=1))
        misc = ctx.enter_context(tc.tile_pool(name="misc", bufs=1))
        tpsum = ctx.enter_context(tc.tile_pool(name="tpsum", bufs=1, space="PSUM"))
        cpsum = ctx.enter_context(tc.tile_pool(name="cpsum", bufs=6, space="PSUM"))
        zpsum = ctx.enter_context(tc.tile_pool(name="zpsum", bufs=1, space="PSUM"))

        # --- constants (sync ring: small, then the 50 g slices) ---
        seq_sb = const.tile([BLOC, L], i32)
        nc.sync.dma_start(out=seq_sb[:], in_=seq_t[:])
        uid_sb = const.tile([BLOC, 1], i32)
        nc.sync.dma_start(out=uid_sb[:], in_=uid_t[:])
        ident = const.tile([128, 128], bf16)
        make_identity(nc, ident[:])
        fcwh_sb = const.tile([128, 7 * D], bf16)
        nc.sync.dma_start(out=fcwh_sb[:], in_=fcwh_t[:])
        mask_sb = const.tile([128, 7 * 512], f32)
        nc.sync.dma_start(out=mask_sb[:], in_=masks_t[:])
        bh_sb = const.tile([128, 7], f32)
        nc.sync.dma_start(out=bh_sb[:], in_=bh_t[:])
        fcb_sb = const.tile([1, D], bf16)
        nc.sync.dma_start(out=fcb_sb[:], in_=fcb_t[:])
        ones_sb = const.tile([1, BLOC], bf16)
        nc.vector.memset(ones_sb[:], 1.0)
        zline = const.tile([D, 1], bf16)
        nc.vector.memset(zline[:], 0.0)
        zfill = const.tile([128, BLOC], bf16)
        nc.vector.memset(zfill[:], 0.0)

        # --- conv weights: one resident tile, streamed per tap-group on the
        # scalar ring, in chase-need order (interleaved into the t-loop) ---
        whp_sb = wpool.tile([D, NWTILES * 128], bf16, name="whp_sb")

        def load_wgroup(g):
            _, u, _, dt0, dt1 = g
            base = CHUNKS[u]["base"]
            c0, c1 = (base + dt0) * 128, (base + dt1) * 128
            nc.scalar.dma_start(out=whp_sb[:, c0:c1], in_=whp_t[:, c0:c1])

        # --- ET[d, b, t'], zero pad for t' >= L ---
        et = etp.tile([D, BLOC, TP], bf16)
        nc.vector.tensor_copy(out=et[:, :, L:TP],
                              in_=zline[:].to_broadcast([D, BLOC, TP - L]))

        # --- fc accumulation PSUM [b, k]; group closes on last o_h matmul.
        # The opening bias matmul is emitted inside the t-loop (after the
        # first transpose) so a slow fcb load can't stall the PE queue head.
        zp = zpsum.tile([BLOC, D], f32)

        # conv emission bookkeeping
        chase = {}
        fc_pending = []
        for u, blk in PHASE_A:
            nt = CHUNKS[u]["nt"]
            for dt in range(CHUNKS[u]["ndt"]):
                chase.setdefault(min(dt + nt - 1, L - 1), []).append((u, blk, dt))
        psum_tiles = {}
        blocks_left = [ch["nblk"] for ch in CHUNKS]
        oh_tiles = {}

        def get_ohu(u):
            if u not in oh_tiles:
                oh_tiles[u] = ohp.tile([128, BLOC], bf16, tag=f"oh{u}",
                                       name=f"oh{u}")
            return oh_tiles[u]

        def emit_conv_mm(u, blk, dt):
            ch = CHUNKS[u]
            nt, nb, ndt = ch["nt"], ch["nb"], ch["ndt"]
            b0 = blk * nb
            nbb = min(nb, BLOC - b0)
            n = nbb * nt
            key = (u, blk)
            if key not in psum_tiles:
                while len(fc_pending) > 1:
                    uu = fc_pending.pop(0)
                    nc.tensor.matmul(out=zp[:], lhsT=oh_tiles[uu][:],
                                     rhs=fcwh_sb[:, uu * D:(uu + 1) * D],
                                     start=False, stop=False)
                psum_tiles[key] = cpsum.tile([128, 512], f32, tag="cps",
                                             name=f"cps_{u}_{blk}")
            ps = psum_tiles[key]
            # Tap dt only feeds heights i >= dt, whose valid t stops at
            # 49 - max(dt, i0): shrink the t-window for late taps. The
            # skipped cells are all masked to -1e30 before the reduce, so
            # the output is bit-identical. Saves ~4% of conv PE rows.
            nt_eff = min(nt, L - max(dt, ch["i0"]))
            out_ap = (ps[:, :n] if nt_eff == nt else
                      ps[:, :n].rearrange("p (b t) -> p b t", t=nt)[:, :, :nt_eff])
            nc.tensor.matmul(
                out=out_ap,
                lhsT=whp_sb[:, (ch["base"] + dt) * 128:(ch["base"] + dt + 1) * 128],
                rhs=et[:, b0:b0 + nbb, dt:dt + nt_eff],
                start=(dt == 0), stop=(dt == ndt - 1))
            if dt == ndt - 1:
                nc.vector.tensor_tensor(
                    out=ps[:, :n], in0=ps[:, :n],
                    in1=mask_sb[:, u * 512:u * 512 + n],
                    op=mybir.AluOpType.add)
                nc.vector.reduce_max(
                    out=get_ohu(u)[:, b0:b0 + nbb],
                    in_=ps[:, :n].rearrange("p (b t) -> p b t", t=nt),
                    axis=X)
                del psum_tiles[key]
                blocks_left[u] -= 1
                if blocks_left[u] == 0:
                    ohu = get_ohu(u)
                    nc.scalar.activation(ohu[:], ohu[:],
                                         mybir.ActivationFunctionType.Relu,
                                         bias=bh_sb[:, u:u + 1])
                    fc_pending.append(u)

        # --- the chase loop: gather -> bf16 cast -> transpose -> copy ->
        # G matmul, with ready conv matmuls interleaved into the PE stream
        # and weight-group DMAs interleaved into the scalar queue ---
        g_tiles = {}

        def emit_g_mm(t):
            nc.tensor.matmul(out=zp[:], lhsT=et[:, :, t], rhs=g_tiles.pop(t),
                             start=False, stop=False)

        for t in range(L):
            for g in WTRIG.get(t, ()):
                load_wgroup(g)
            e_t = egath.tile([BLOC, D], f32, tag="eg")
            nc.gpsimd.indirect_dma_start(
                out=e_t[:], out_offset=None, in_=item_t[:],
                in_offset=IndirectOffsetOnAxis(ap=seq_sb[:, t:t + 1], axis=0))
            ebf = ebfp.tile([BLOC, D], bf16, tag="ebf")
            nc.scalar.activation(ebf[:], e_t[:], Copy)
            tp = tpsum.tile([128, 128], bf16, tag="tp")
            nc.tensor.transpose(out=tp[:], in_=ebf[:], identity=ident[:])
            nc.vector.tensor_copy(out=et[:, :, t], in_=tp[:])
            gt = gpool.tile([D, D], bf16, tag="g", name=f"g{t}")
            nc.sync.dma_start(out=gt[:], in_=g_t[:, t * D:(t + 1) * D])
            g_tiles[t] = gt
            if t == 0:
                nc.tensor.matmul(out=zp[:], lhsT=ones_sb[:], rhs=fcb_sb[:],
                                 start=True, stop=False)
            else:
                emit_g_mm(t - 1)
            for (u, blk, dt) in chase.get(t, ()):
                emit_conv_mm(u, blk, dt)
            # Zero-contribution fillers (exact +0 into the open zp group)
            # keep the PE gaplessly busy through the gather window so HAM
            # ramps to full clock early and stays there.
            if t >= 1:
                nfill = 12 if t < 9 else (6 if t < 17 else 4)
                for _ in range(nfill):
                    nc.tensor.matmul(out=zp[:], lhsT=zfill[:],
                                     rhs=ident[:], start=False, stop=False)
        emit_g_mm(L - 1)

        # --- the P_u gather (off the gather window's critical path) ---
        pu_sb = misc.tile([BLOC, D], f32, tag="pu")
        nc.gpsimd.indirect_dma_start(
            out=pu_sb[:], out_offset=None, in_=user_t[:],
            in_offset=IndirectOffsetOnAxis(ap=uid_sb[:, :1], axis=0))
        nc.sync.dma_start(out=out_t[:, D:NOUT], in_=pu_sb[:])

        # --- remaining conv chunks, block-sequential: each block's reduce
        # overlaps the next block's matmuls on a different PSUM bank ---
        done_a = set(PHASE_A)
        for u in [4, 3, 2, 1, 0]:
            for blk in range(CHUNKS[u]["nblk"]):
                if (u, blk) in done_a:
                    continue
                for dt in range(CHUNKS[u]["ndt"]):
                    emit_conv_mm(u, blk, dt)

        # --- remaining o_h fc matmuls ---
        for i, u in enumerate(fc_pending):
            nc.tensor.matmul(out=zp[:], lhsT=oh_tiles[u][:],
                             rhs=fcwh_sb[:, u * D:(u + 1) * D],
                             start=False, stop=(i == len(fc_pending) - 1))

        z_sb = misc.tile([BLOC, D], f32, tag="z")
        nc.scalar.activation(z_sb[:], zp[:], mybir.ActivationFunctionType.Relu)
        nc.sync.dma_start(out=out_t[:, 0:D], in_=z_sb[:])

    return nc


def _prep_common(user_emb, item_emb, Wv, bv, Wh, bh, fc_W, fc_b):
    f = np.float32
    b16 = ml_dtypes.bfloat16
    item_emb = np.ascontiguousarray(np.asarray(item_emb, f))
    user_emb = np.ascontiguousarray(np.asarray(user_emb, f))
    Wh = np.asarray(Wh, f)          # [L, NH, L, D], zero for dt > i
    Wv = np.asarray(Wv, f)          # [NV, L]
    bv = np.asarray(bv, f)
    bh = np.asarray(bh, f)          # [L, NH]
    fc_W = np.asarray(fc_W, f)      # [FC_IN, D]
    fc_b = np.asarray(fc_b, f)

    whp = np.zeros((D, NWTILES * 128), f)
    masks = np.full((128, 7 * 512), 0.0, f)
    fcwh = np.zeros((128, 7 * D), f)
    bh_p = np.zeros((128, 7), f)
    fcw_h = fc_W[NV * D:]           # [800, D]
    for u, ch in enumerate(CHUNKS):
        i0, nh, ndt, nt, nb = ch["i0"], ch["nh"], ch["ndt"], ch["nt"], ch["nb"]
        base = ch["base"]
        wu = Wh[i0:i0 + nh]         # [nh, NH, L, D]
        for dt in range(ndt):
            blkw = wu[:, :, dt, :].reshape(nh * NH, D)
            whp[:, (base + dt) * 128:(base + dt) * 128 + nh * NH] = blkw.T
        m = np.full((128, nb * nt), NEG, f)
        for mm in range(nh * NH):
            i = i0 + mm // NH
            vt = min(L - i, nt)
            row = np.full((nt,), NEG, f)
            row[:vt] = 0.0
            m[mm] = np.tile(row, nb)
        masks[:, u * 512:u * 512 + nb * nt] = m
        fcwh[:nh * NH, u * D:(u + 1) * D] = fcw_h[u * 128:u * 128 + nh * NH]
        bh_p[:nh * NH, u] = bh[i0:i0 + nh].reshape(nh * NH)

    fcv = fc_W[:NV * D].reshape(NV, D, D)
    g = np.einsum("vt,vdk->tdk", Wv, fcv)            # [L, D, D]
    g = np.ascontiguousarray(g.transpose(1, 0, 2).reshape(D, L * D))
    fcb = (fc_b + np.einsum("v,vdk->k", bv, fcv)).reshape(1, D)

    return dict(item_emb=item_emb, user_emb=user_emb,
                whp=whp.astype(b16), g=g.astype(b16),
                fcwh=fcwh.astype(b16), masks=masks, bh_p=bh_p,
                fcb=fcb.astype(b16))


def make_in_maps(user_ids, item_seq, user_emb, item_emb, Wv, bv, Wh, bh,
                 fc_W, fc_b):
    common = _prep_common(user_emb, item_emb, Wv, bv, Wh, bh, fc_W, fc_b)
    user_ids = np.asarray(user_ids).astype(np.int32).reshape(B, 1)
    item_seq = np.asarray(item_seq).astype(np.int32).reshape(B, L)
    in_maps = []
    for c in range(NCORES):
        m = dict(common)
        m["seq_idx"] = np.ascontiguousarray(item_seq[c * BLOC:(c + 1) * BLOC])
        m["uid_idx"] = np.ascontiguousarray(user_ids[c * BLOC:(c + 1) * BLOC])
        in_maps.append(m)
    return in_maps


def get_nc():
    global _NC_CACHE
    if _NC_CACHE is None:
        _NC_CACHE = _build_nc()
        _install_wait_splitter(_NC_CACHE)
    return _NC_CACHE


def kernel(**inputs) -> np.ndarray:
    global LAST_RESULTS
    in_maps = make_in_maps(**inputs)
    nc = get_nc()
    trace = bool(int(os.environ.get("BASS_KERNEL_TRACE", "0")))
    res = run_bass_kernel_spmd(nc, in_maps, list(range(NCORES)), trace=trace)
    LAST_RESULTS = res
    return np.concatenate([res.results[c]["out"] for c in range(NCORES)], axis=0)



# revision 37
# speedup vs baseline: 1.7357x; 1.7357x over previous
"""Trainium2 Bass kernel for nn_CaserQueryEncoder.

Model (B=1024, L=50, D=128, NV=8, NH=16):
  P_u = user_emb[user_ids]                                   [B, D]
  E   = item_emb[item_seq]                                   [B, L, D]
  o_v = einsum('btd,vt->bvd', E, Wv) + bv                    [B, NV*D]
  conv[b,i,j,t] = sum_{dt<=i} <E[b, t+dt, :], Wh[i,j,dt,:]>  (Wh zero for dt>i)
  o_h[b,i,j] = max over valid t (t <= 49-i) of relu(conv + bh)
  z = relu([o_v, o_h] @ fc_W + fc_b)                         [B, D]
  out = [z, P_u]                                             [B, 2D]

Strategy: pure data parallel, 128 batch rows per core x 8 cores, no
collectives. Per core:
  - E gathered in 7 batched indirect DMAs (8 cols each) instead of 50
    single-column ones: the ~1us SWDGE fixed cost per op dominated the
    gather window.  Each batch is cast to bf16 with a x64 scale on the
    scalar engine and PE-transposed into t-major tiles
      et  [d, t', b]       bf16   (rhs for the bf16 conv chunk u5)
      et8 [d, 2, t', b]    fp8e4  (slot i holds E[t'+i]*64; rhs for
                                   DoubleRow fp8 matmuls)
    t-major makes every matmul rhs a contiguous [128, (2,) nt*128] AP
    over full batch columns and the DVE fan-out copies contiguous.
  - horizontal conv: heights packed 8 per chunk (x16 filters = 128
    weight columns), PSUM blocked over t (4 t's x 128 b = 512 cols).
    Chunks u0-u4 and u6 run as fp8e4 DoubleRow matmuls: taps (2p, 2p+1)
    share one pass, K=2x128, 2 MACs per PE cell per cycle - half the PE
    columns of bf16.  Chunk u5 (heights 40..47, the dominant error
    contributors) stays bf16.  All operands carry x64, so PSUM holds
    4096*conv; bias/fc-side constants are pre-scaled by 4096 on host and
    the final ReLU divides it out.  Quantization rel-err vs the fp32
    reference: ~1.5e-2 (gate 2e-2); bf16-only was 2.3e-3.
  - a pass is emitted for t-block [t0,t0+tb) only when t0 < nt_eff(dt);
    cells where some taps were skipped or some heights are past their
    valid window live only in the last 2 t-blocks of each chunk and get
    a -1e30 mask row (broadcast over b) before the max-reduce, so
    outputs are unchanged.
  - per t-block: reduce_max over t via a strided [d, b, t] view, then a
    running elementwise max into oh[u].
  - vertical conv never materialized: G[t,d,k] = sum_v Wv[v,t]*fc_W[..]
    is precomputed on host, quantized to fp8, and accumulated into the
    fc PSUM as 25 DoubleRow passes over (t, t+1) pairs.
  - conv weights stream as need-ordered groups on the scalar ring so
    chase blocks (u6, u5, u4 tblks 0-1) run inside the gather window;
    zero-contribution filler matmuls bridge the first ~4us and keep the
    PE HAM clock-gate warm.
"""

import math
import os
import sys
from collections import defaultdict
from contextlib import ExitStack

import numpy as np

sys.path.insert(0, "/opt/trn_rl_repo")

import ml_dtypes

import concourse.bass as bass
import concourse.tile as tile
from concourse import mybir
from concourse.bass import IndirectOffsetOnAxis
from concourse.bass_utils import run_bass_kernel_spmd
from concourse.masks import make_identity
from concourse.vector_clock import ScopedClock


def _patch_tile_drain():
    """This container's walrus codegen only accepts one sync-wait per Drain
    (CTRL_NO_STRUCT); Tile's kernel-tail drain carries one wait per live
    semaphore. Split the waits across a chain of drains, one wait each."""
    if getattr(tile.TileContext, "_drain_split_patched", False):
        return

    def _patched(self, tick_clock, wait_clock):
        nc = self.nc
        probe = nc.sync.drain()
        wait_clock.add_sem_waits(
            probe.ins, ScopedClock({None: tick_clock.global_clock}))
        nc.all_engine_barrier()
        popped = nc._tile_sem_poison_stack.pop()
        assert popped is self._sem_poison
        nc.clear_and_free_semaphores(list(self.sems.allocated().values()))
        nc.all_engine_barrier()

    tile.TileContext._drain_and_barrier = _patched
    tile.TileContext._drain_split_patched = True


_patch_tile_drain()


def _split_json_waits(j, max_waits=1):
    """This walrus codegen accepts at most one sync-wait per instruction.
    Hoist extra waits onto wait-only EventSemaphore instructions inserted
    just before the offender on the same engine queue."""
    n = 0
    for fn in j["functions"]:
        for blk in fn["blocks"]:
            out = []
            for inst in blk["instructions"]:
                si = inst.get("sync_info")
                waits = (si or {}).get("on_wait") or []
                if len(waits) > max_waits:
                    for k, w in enumerate(waits[:-max_waits]):
                        out.append({
                            "debug": inst.get("debug", 0),
                            "engine": inst["engine"],
                            "ins": [], "outs": [],
                            "name": f"{inst['name']}_wsplit{k}",
                            "opcode": "EventSemaphore",
                            "sync_info": {"on_update": [], "on_wait": [w]},
                        })
                        n += 1
                    si["on_wait"] = waits[-max_waits:]
                out.append(inst)
            blk["instructions"] = out
    return n


def _install_wait_splitter(nc):
    import json as _json

    orig = nc.to_json_bytes

    def patched():
        j = _json.loads(orig())
        _split_json_waits(j)
        return _json.dumps(j).encode()

    nc.to_json_bytes = patched

B = 1024
L = 50
D = 128
NV = 8
NH = 16
NU = 100000
NI = 500000
NCORES = 8
BLOC = B // NCORES          # 128 batch rows per core
TP = 54                     # et bf16 t' pitch (u5 taps read up to t'=53)
TP8 = 52                    # et8 t' pitch; fp8 slices end at t'<=52
NEG = -1.0e30
SCALE = 64.0                # E and conv weights carry x64 -> PSUM x4096
SC2 = SCALE * SCALE
NOUT = 2 * D                # 256
TBLK = 4                    # t's per conv PSUM tile (4*128 b = 512 cols)

FP8U = tuple(int(x) for x in os.environ.get(
    "CASER_FP8U", "0,1,2,3,4,6").split(",") if x != "")
G_FP8 = os.environ.get("CASER_G_FP8", "1") == "1"

# Height-chunk table: heights [8u, 8u+nh) packed as m2 = 16*(i-8u)+j.
CHUNKS = []
_b8 = 0
_b16 = 0
for _u in range(7):
    _i0 = 8 * _u
    _nh = min(8, L - _i0)
    _ndt = min(_i0 + 8, L)
    _nt = L - _i0
    _fp8 = _u in FP8U
    _np8 = _ndt // 2 if _fp8 else 0
    CHUNKS.append(dict(i0=_i0, nh=_nh, ndt=_ndt, nt=_nt,
                       ntb=math.ceil(_nt / TBLK), fp8=_fp8,
                       npass=_np8, base=(_b8 if _fp8 else _b16)))
    _b8 += _np8
    if not _fp8:
        _b16 += _ndt
NT8 = _b8                   # fp8 pass tiles (two [d, 128] slot blocks each)
NT16 = max(_b16, 1)         # bf16 tap tiles (each [d, 128])
NG = L // 2                 # G DoubleRow passes


def _nt_eff(u, dt):
    ch = CHUNKS[u]
    return min(ch["nt"], L - max(dt, ch["i0"]))


def _tb_of(u, tblk):
    t0 = tblk * TBLK
    return t0, min(TBLK, CHUNKS[u]["nt"] - t0)


def _tbe_fp8(u, tblk, p):
    # effective t's of pass (2p, 2p+1) on t-block [t0, t0+tb): trimmed to
    # the tap's valid window (pass 0 always covers the full block)
    t0, tb = _tb_of(u, tblk)
    return min(tb, _nt_eff(u, 2 * p) - t0)


def _tbe_bf16(u, tblk, dt):
    t0, tb = _tb_of(u, tblk)
    return min(tb, _nt_eff(u, dt) - t0)


def _need_fp8(u, tblk, p):
    # slot1 reads real cols up to min(2p + t0 + tbe, 49); t' beyond 49 is
    # the zero pad.
    t0, _ = _tb_of(u, tblk)
    return min(2 * p + t0 + _tbe_fp8(u, tblk, p), L - 1)


def _need_bf16(u, tblk, dt):
    t0, _ = _tb_of(u, tblk)
    return min(dt + t0 + _tbe_bf16(u, tblk, dt) - 1, L - 1)


# Gather batches: [t0, t1) column ranges, one indirect DMA each.  The HW
# SWDGE consumes exactly one offset per partition per op (measured: a
# [128, k] offset AP gathers k*128 elements CONTIGUOUS from row idx[b,0]),
# so batches must be single columns.
GBW = int(os.environ.get("CASER_GBW", "1"))
GBATCH = [(t0, min(t0 + GBW, L)) for t0 in range(0, L, GBW)]
NBATCH = len(GBATCH)


def _batch_of(col):
    return min(col // GBW, NBATCH - 1)


# --- chase schedule -----------------------------------------------------
# The gather runs ~50 single-column ops at ~1.2us each, so the window is
# ~60us: it must be filled with real conv work.  Six "cursors" (= the 6
# cpsum bufs) walk their chunk's t-blocks sequentially; a cursor emits a
# t-block's passes as their needed columns land, closes it, then advances.
# u5's three t-blocks each get their own cursor (their taps spread over
# the whole window); u2/u1/u0 ladder through their t-blocks (early blocks
# close early and recycle the slot).  u6/u4/u3 + leftovers run post-gather.


def _tblk_items(u, tblk):
    """(need, kind, u, tblk, idx) for every emitted pass of one t-block."""
    ch = CHUNKS[u]
    t0, _ = _tb_of(u, tblk)
    out = []
    if ch["fp8"]:
        for p in range(ch["npass"]):
            if t0 < _nt_eff(u, 2 * p):
                out.append((_need_fp8(u, tblk, p), "c8", u, tblk, p))
    else:
        for dt in range(ch["ndt"]):
            if t0 < _nt_eff(u, dt):
                out.append((_need_bf16(u, tblk, dt), "c16", u, tblk, dt))
    return out


CHASE_CURSORS = [[(5, 0)], [(5, 1)], [(5, 2)],
                 [(2, k) for k in range(CHUNKS[2]["ntb"])],
                 [(1, k) for k in range(CHUNKS[1]["ntb"])],
                 [(0, k) for k in range(CHUNKS[0]["ntb"])]]
CHASE_BY_BATCH = defaultdict(list)
CHASED_TBLKS = set()
_cursor_pos = [0] * len(CHASE_CURSORS)
_cursor_items = [None] * len(CHASE_CURSORS)
for _t in range(L):
    for _ci, _blklist in enumerate(CHASE_CURSORS):
        while True:
            if _cursor_items[_ci] is None:
                if _cursor_pos[_ci] >= len(_blklist):
                    break
                _ut = _blklist[_cursor_pos[_ci]]
                _cursor_items[_ci] = _tblk_items(*_ut)
                CHASED_TBLKS.add(_ut)
            items = _cursor_items[_ci]
            ready = [it for it in items if it[0] <= _t]
            for it in ready:
                CHASE_BY_BATCH[_batch_of(_t)].append(it)
                items.remove(it)
            if not items:
                _cursor_items[_ci] = None
                _cursor_pos[_ci] += 1
                continue
            break
# at _t = 49 every need is <= 49, so each cursor fully drains its chunk:
# all cursor-chunk t-blocks are in CHASE_BY_BATCH and CHASED_TBLKS.
assert all(it is None for it in _cursor_items)

# post-gather chunks, u6 last so the final close chain is one small block
POST_CHUNKS = [4, 3, 6]

# G passes: pass p reads et8 slot1 col 2p (=E[2p+1]); +8 col lag so the
# g8 weight DMA has surely landed before the PE hits the matmul.
G_BY_BATCH = defaultdict(list)
if G_FP8:
    for _p in range(NG):
        G_BY_BATCH[_batch_of(min(2 * _p + 1 + 8, L - 1))].append(_p)
else:
    for _p in range(L):
        G_BY_BATCH[_batch_of(min(_p + 8, L - 1))].append(_p)

# Weight-DMA groups (<=8 tiles each), triggered a few cols before first
# possible use (t-block 0): (trigger_col, first_need, kind, u, lo, hi)
WGROUPS = []
for _u, _ch in enumerate(CHUNKS):
    if _ch["fp8"]:
        needs = [_need_fp8(_u, 0, p) for p in range(_ch["npass"])]
        kind, n = "w8", _ch["npass"]
    else:
        needs = [_need_bf16(_u, 0, dt) for dt in range(_ch["ndt"])]
        kind, n = "w16", _ch["ndt"]
    for lo in range(0, n, 8):
        hi = min(lo + 8, n)
        first = min(needs[lo:hi])
        WGROUPS.append((max(0, first - 6), first, kind, _u, lo, hi))
WG_BY_BATCH = defaultdict(list)
for _g in sorted(WGROUPS):
    WG_BY_BATCH[_batch_of(_g[0])].append(_g)

# filler quotas: bridge kernel start -> first real PE work, then fill the
# per-column deficit between available chase work and the gather cadence
FILL_PRE = 50
_COL_BUDGET_US = 1.25       # ~gather cadence per column
_FILL_US = 0.0536           # one 128-col bf16 filler at warm clock


def _item_cost_us(it):
    _, kind, u, tblk, idx = it
    tbe = _tbe_fp8(u, tblk, idx) if kind == "c8" else _tbe_bf16(u, tblk, idx)
    return tbe * (0.0559 if kind == "c8" else 0.112)


FILL_BATCH = {}
for _t in range(L):
    if _t >= 44:
        FILL_BATCH[_t] = 0
        continue
    w = 0.056 + sum(_item_cost_us(it) for it in CHASE_BY_BATCH.get(_t, ()))
    w += 0.056 * len(G_BY_BATCH.get(_t, ()))
    FILL_BATCH[_t] = max(0, min(22, int((_COL_BUDGET_US - w) / _FILL_US)))

_NC_CACHE = None

# Set BASS_KERNEL_TRACE=1 to profile; exec time lands in LAST_RESULTS.
LAST_RESULTS = None


def _build_nc():
    f32 = mybir.dt.float32
    bf16 = mybir.dt.bfloat16
    f8 = mybir.dt.float8e4
    i32 = mybir.dt.int32
    X = mybir.AxisListType.X
    Copy = mybir.ActivationFunctionType.Copy
    Relu = mybir.ActivationFunctionType.Relu
    DR = mybir.MatmulPerfMode.DoubleRow
    Max = mybir.AluOpType.max

    nc = bass.Bass()
    seq_t = nc.dram_tensor("seq_idx", [BLOC, L], i32, kind="ExternalInput")
    uid_t = nc.dram_tensor("uid_idx", [BLOC, 1], i32, kind="ExternalInput")
    item_t = nc.dram_tensor("item_emb", [NI, D], f32, kind="ExternalInput")
    user_t = nc.dram_tensor("user_emb", [NU, D], f32, kind="ExternalInput")
    whp8_t = nc.dram_tensor("whp8", [D, max(2 * NT8, 1) * 128], f8,
                            kind="ExternalInput")
    whp16_t = nc.dram_tensor("whp16", [D, NT16 * 128], bf16,
                             kind="ExternalInput")
    if G_FP8:
        g8_t = nc.dram_tensor("g8", [D, 2 * NG * 128], f8,
                              kind="ExternalInput")
    else:
        g8_t = nc.dram_tensor("g8", [D, L * D], bf16, kind="ExternalInput")
    fcwh_t = nc.dram_tensor("fcwh", [128, 7 * D], bf16, kind="ExternalInput")
    mrow_t = nc.dram_tensor("mrow", [128, 7 * 64, 1], f32,
                            kind="ExternalInput")
    bh_t = nc.dram_tensor("bh_p", [128, 7], f32, kind="ExternalInput")
    fcb_t = nc.dram_tensor("fcb", [1, D], bf16, kind="ExternalInput")
    out_t = nc.dram_tensor("out", [BLOC, NOUT], f32, kind="ExternalOutput")

    with ExitStack() as ctx:
        tc = ctx.enter_context(tile.TileContext(nc))
        const = ctx.enter_context(tc.tile_pool(name="const", bufs=1))
        egath = ctx.enter_context(tc.tile_pool(name="egath", bufs=1))
        ebfp = ctx.enter_context(tc.tile_pool(name="ebfp", bufs=2))
        etp = ctx.enter_context(tc.tile_pool(name="etp", bufs=1))
        wpool = ctx.enter_context(tc.tile_pool(name="wpool", bufs=1))
        ohp = ctx.enter_context(tc.tile_pool(name="ohp", bufs=1))
        tmpp = ctx.enter_context(tc.tile_pool(name="tmpp", bufs=2))
        misc = ctx.enter_context(tc.tile_pool(name="misc", bufs=1))
        tpsum = ctx.enter_context(tc.tile_pool(name="tpsum", bufs=1, space="PSUM"))
        cpsum = ctx.enter_context(tc.tile_pool(name="cpsum", bufs=6, space="PSUM"))
        zpsum = ctx.enter_context(tc.tile_pool(name="zpsum", bufs=1, space="PSUM"))

        # --- constants; sync ring order = landing order ---
        seq_sb = const.tile([BLOC, L], i32)
        nc.sync.dma_start(out=seq_sb[:], in_=seq_t[:])
        fcb_sb = const.tile([1, D], bf16)
        nc.sync.dma_start(out=fcb_sb[:], in_=fcb_t[:])
        uid_sb = const.tile([BLOC, 1], i32)
        nc.sync.dma_start(out=uid_sb[:], in_=uid_t[:])
        g8_sb = const.tile([D, 2 * NG * 128], f8 if G_FP8 else bf16)
        nc.sync.dma_start(out=g8_sb[:, :NG * 128], in_=g8_t[:, :NG * 128])
        nc.sync.dma_start(out=g8_sb[:, NG * 128:], in_=g8_t[:, NG * 128:])
        fcwh_sb = const.tile([128, 7 * D], bf16)
        nc.sync.dma_start(out=fcwh_sb[:], in_=fcwh_t[:])
        bh_sb = const.tile([128, 7], f32)
        nc.sync.dma_start(out=bh_sb[:], in_=bh_t[:])
        mrow_sb = const.tile([128, 7 * 64, 1], f32)
        nc.sync.dma_start(out=mrow_sb[:], in_=mrow_t[:])

        ident = const.tile([128, 128], bf16)
        make_identity(nc, ident[:])
        ones_sb = const.tile([1, BLOC], bf16)
        nc.vector.memset(ones_sb[:], 1.0)
        zfill = const.tile([128, BLOC], bf16)
        nc.vector.memset(zfill[:], 0.0)

        # --- ET tiles (t-major) ---
        et = etp.tile([D, TP, BLOC], bf16, name="et")
        et8 = etp.tile([D, 2, TP8, BLOC], f8, name="et8")
        # zero only the pad rows the conv can read past t'=49 (overshoot
        # cells are masked before the reduce, but reads must be finite)
        nc.vector.memset(et8[:, :, L - 1:, :], 0.0)
        nc.vector.memset(et[:, L:, :], 0.0)

        g8_3 = g8_sb[:].rearrange("d (i r) -> d i r", i=2)

        # --- conv weights: resident tiles, streamed per group ---
        whp8_sb = wpool.tile([D, max(2 * NT8, 1) * 128], f8, name="whp8_sb")
        whp16_sb = wpool.tile([D, NT16 * 128], bf16, name="whp16_sb")
        whp8_3 = (whp8_sb[:].rearrange("d (i r) -> d i r", i=2)
                  if NT8 else None)

        def load_wgroup(g):
            _, _, kind, u, lo, hi = g
            if kind == "w8":
                base = CHUNKS[u]["base"]
                c0, c1 = (base + lo) * 128, (base + hi) * 128
                nc.scalar.dma_start(out=whp8_sb[:, c0:c1],
                                    in_=whp8_t[:, c0:c1])
                off = NT8 * 128
                nc.scalar.dma_start(out=whp8_sb[:, off + c0:off + c1],
                                    in_=whp8_t[:, off + c0:off + c1])
            else:
                base = CHUNKS[u]["base"]
                c0, c1 = (base + lo) * 128, (base + hi) * 128
                nc.scalar.dma_start(out=whp16_sb[:, c0:c1],
                                    in_=whp16_t[:, c0:c1])

        # --- the 7 batched E gathers, all queued upfront on gpsimd ---
        e_tiles = []
        for bi, (t0, t1) in enumerate(GBATCH):
            k = t1 - t0
            e_t = egath.tile([BLOC, k * D], f32, name=f"eg{bi}")
            nc.gpsimd.indirect_dma_start(
                out=e_t[:], out_offset=None, in_=item_t[:],
                in_offset=IndirectOffsetOnAxis(ap=seq_sb[:, t0:t1], axis=0))
            e_tiles.append(e_t)
        # P_u gather once the E stream is queued
        pu_sb = misc.tile([BLOC, D], f32, name="pu")
        nc.gpsimd.indirect_dma_start(
            out=pu_sb[:], out_offset=None, in_=user_t[:],
            in_offset=IndirectOffsetOnAxis(ap=uid_sb[:, :1], axis=0))
        nc.sync.dma_start(out=out_t[:, D:NOUT], in_=pu_sb[:])

        # --- all conv-weight groups issued upfront in need order: the
        # scalar ring streams them while the casts (which wait on the
        # gathers) would otherwise block the queue ---
        for bi in range(NBATCH):
            for g in WG_BY_BATCH.get(bi, ()):
                load_wgroup(g)

        # --- fc accumulation PSUM [b, k]: group = opening filler (zeros,
        # start=True), fillers, bias, 25 G passes, 7 o_h matmuls (stop).
        zp = zpsum.tile([BLOC, D], f32)
        zp_opened = [False]

        def filler():
            nc.tensor.matmul(out=zp[:], lhsT=zfill[:], rhs=ident[:],
                             start=not zp_opened[0], stop=False)
            zp_opened[0] = True

        for _ in range(FILL_PRE):
            filler()

        # --- conv emission machinery ---
        psum_tiles = {}
        tblks_left = [ch["ntb"] for ch in CHUNKS]
        oh_tiles = {}
        fc_pending = []

        def open_psum(u, tblk):
            key = (u, tblk)
            if key not in psum_tiles:
                while len(fc_pending) > 1:
                    uu = fc_pending.pop(0)
                    nc.tensor.matmul(out=zp[:], lhsT=oh_tiles[uu][:],
                                     rhs=fcwh_sb[:, uu * D:(uu + 1) * D],
                                     start=False, stop=False)
                psum_tiles[key] = cpsum.tile([128, 512], f32, tag="cps",
                                             name=f"cps_{u}_{tblk}")
            return psum_tiles[key]

        def close_psum(u, tblk):
            ch = CHUNKS[u]
            t0, tb = _tb_of(u, tblk)
            n = tb * BLOC
            ps = psum_tiles[(u, tblk)]
            # cells with a skipped tap or past a height's valid window only
            # exist for t >= nt-7: mask the last two t-blocks
            if t0 + tb > ch["nt"] - 7:
                m0 = u * 64 + t0
                nc.vector.tensor_tensor(
                    out=ps[:, :n].rearrange("p (t b) -> p t b", t=tb),
                    in0=ps[:, :n].rearrange("p (t b) -> p t b", t=tb),
                    in1=mrow_sb[:, m0:m0 + tb, 0:1].to_broadcast(
                        [128, tb, BLOC]),
                    op=mybir.AluOpType.add)
            first = (tblks_left[u] == ch["ntb"])
            ohu = get_ohu(u)
            if first:
                nc.vector.reduce_max(
                    out=ohu[:],
                    in_=ps[:, :n].rearrange("p (t b) -> p b t", b=BLOC),
                    axis=X)
            else:
                tmp = tmpp.tile([128, BLOC], bf16, tag="redtmp")
                nc.vector.reduce_max(
                    out=tmp[:],
                    in_=ps[:, :n].rearrange("p (t b) -> p b t", b=BLOC),
                    axis=X)
                nc.vector.tensor_tensor(out=ohu[:], in0=ohu[:], in1=tmp[:],
                                        op=Max)
            del psum_tiles[(u, tblk)]
            tblks_left[u] -= 1
            if tblks_left[u] == 0:
                nc.scalar.activation(ohu[:], ohu[:], Relu,
                                     bias=bh_sb[:, u:u + 1])
                fc_pending.append(u)

        def get_ohu(u):
            if u not in oh_tiles:
                oh_tiles[u] = ohp.tile([128, BLOC], bf16, name=f"oh{u}")
            return oh_tiles[u]

        def last_fp8_pass(u, tblk):
            t0, _ = _tb_of(u, tblk)
            return max(p for p in range(CHUNKS[u]["npass"])
                       if t0 < _nt_eff(u, 2 * p))

        def last_bf16_tap(u, tblk):
            t0, _ = _tb_of(u, tblk)
            return max(dt for dt in range(CHUNKS[u]["ndt"])
                       if t0 < _nt_eff(u, dt))

        def emit_fp8_pass(u, tblk, p):
            ch = CHUNKS[u]
            t0, tb = _tb_of(u, tblk)
            tbe = _tbe_fp8(u, tblk, p)
            ps = open_psum(u, tblk)
            base = ch["base"]
            nc.tensor.matmul(
                out=ps[:, :tbe * BLOC],
                lhsT=whp8_3[:, :, (base + p) * 128:(base + p + 1) * 128],
                rhs=et8[:, :, 2 * p + t0:2 * p + t0 + tbe, :],
                start=(p == 0), stop=(p == last_fp8_pass(u, tblk)),
                perf_mode=DR)
            if p == last_fp8_pass(u, tblk):
                close_psum(u, tblk)

        def emit_bf16_tap(u, tblk, dt):
            t0, tb = _tb_of(u, tblk)
            tbe = _tbe_bf16(u, tblk, dt)
            ps = open_psum(u, tblk)
            base = CHUNKS[u]["base"]
            nc.tensor.matmul(
                out=ps[:, :tbe * BLOC],
                lhsT=whp16_sb[:, (base + dt) * 128:(base + dt + 1) * 128],
                rhs=et[:, dt + t0:dt + t0 + tbe, :],
                start=(dt == 0), stop=(dt == last_bf16_tap(u, tblk)))
            if dt == last_bf16_tap(u, tblk):
                close_psum(u, tblk)

        def emit_g_pass(p):
            if G_FP8:
                nc.tensor.matmul(
                    out=zp[:],
                    lhsT=et8[:, :, 2 * p, :],
                    rhs=g8_3[:, :, p * 128:(p + 1) * 128],
                    start=False, stop=False, perf_mode=DR)
            else:
                nc.tensor.matmul(
                    out=zp[:], lhsT=et[:, p, :],
                    rhs=g8_sb[:, p * 128:(p + 1) * 128],
                    start=False, stop=False)

        # --- the batch loop: process gathered columns, chase conv work ---
        for bi, (t0, t1) in enumerate(GBATCH):
            k = t1 - t0
            ebf = ebfp.tile([BLOC, k * D], bf16, tag="ebf")
            nc.scalar.activation(ebf[:], e_tiles[bi][:], Copy, scale=SCALE)
            tp = tpsum.tile([128, k * 128], bf16, tag="tp")
            for tr in range(k):
                nc.tensor.transpose(out=tp[:, tr * 128:(tr + 1) * 128],
                                    in_=ebf[:, tr * 128:(tr + 1) * 128],
                                    identity=ident[:])
            # tp is [d, (t b)] == the t-major layout: contiguous copies
            nc.vector.tensor_copy(out=et[:, t0:t1, :], in_=tp[:])
            nc.vector.tensor_copy(out=et8[:, 0, t0:t1, :], in_=tp[:])
            # slot1[c] = E[c+1]: shift slot0 down by one t'
            s0 = max(t0 - 1, 0)
            if t1 - 1 > s0:
                nc.vector.tensor_copy(out=et8[:, 1, s0:t1 - 1, :],
                                      in_=et8[:, 0, s0 + 1:t1, :])
            if bi == 0:
                nc.tensor.matmul(out=zp[:], lhsT=ones_sb[:], rhs=fcb_sb[:],
                                 start=False, stop=False)
            for _ in range(FILL_BATCH.get(bi, 0)):
                filler()
            for p in G_BY_BATCH.get(bi, ()):
                emit_g_pass(p)
            for item in CHASE_BY_BATCH.get(bi, ()):
                _, kind, u, tblk, idx = item
                if kind == "c8":
                    emit_fp8_pass(u, tblk, idx)
                else:
                    emit_bf16_tap(u, tblk, idx)

        # --- remaining conv t-blocks, block-sequential ---
        for u in POST_CHUNKS:
            ch = CHUNKS[u]
            for tblk in range(ch["ntb"]):
                if (u, tblk) in CHASED_TBLKS:
                    continue
                t0, tb = _tb_of(u, tblk)
                if ch["fp8"]:
                    for p in range(ch["npass"]):
                        if t0 < _nt_eff(u, 2 * p):
                            emit_fp8_pass(u, tblk, p)
                else:
                    for dt in range(ch["ndt"]):
                        if t0 < _nt_eff(u, dt):
                            emit_bf16_tap(u, tblk, dt)

        # --- remaining o_h fc matmuls close the zp group ---
        for i, u in enumerate(fc_pending):
            nc.tensor.matmul(out=zp[:], lhsT=oh_tiles[u][:],
                             rhs=fcwh_sb[:, u * D:(u + 1) * D],
                             start=False, stop=(i == len(fc_pending) - 1))

        z_sb = misc.tile([BLOC, D], f32, name="z")
        nc.scalar.activation(z_sb[:], zp[:], Relu, scale=1.0 / SC2)
        nc.sync.dma_start(out=out_t[:, 0:D], in_=z_sb[:])

    return nc


def _prep_common(user_emb, item_emb, Wv, bv, Wh, bh, fc_W, fc_b):
    f = np.float32
    b16 = ml_dtypes.bfloat16
    f8 = ml_dtypes.float8_e4m3
    item_emb = np.ascontiguousarray(np.asarray(item_emb, f))
    user_emb = np.ascontiguousarray(np.asarray(user_emb, f))
    Wh = np.asarray(Wh, f)          # [L, NH, L, D], zero for dt > i
    Wv = np.asarray(Wv, f)          # [NV, L]
    bv = np.asarray(bv, f)
    bh = np.asarray(bh, f)          # [L, NH]
    fc_W = np.asarray(fc_W, f)      # [FC_IN, D]
    fc_b = np.asarray(fc_b, f)

    whp8 = np.zeros((D, max(2 * NT8, 1) * 128), f)
    whp16 = np.zeros((D, NT16 * 128), f)
    fcwh = np.zeros((128, 7 * D), f)
    mrow = np.zeros((128, 7 * 64, 1), f)
    bh_p = np.zeros((128, 7), f)
    fcw_h = fc_W[NV * D:]           # [800, D]
    for u, ch in enumerate(CHUNKS):
        i0, nh, ndt, nt = ch["i0"], ch["nh"], ch["ndt"], ch["nt"]
        wu = Wh[i0:i0 + nh] * SCALE     # [nh, NH, L, D]
        base = ch["base"]
        if ch["fp8"]:
            for p in range(ch["npass"]):
                for i in range(2):
                    blkw = wu[:, :, 2 * p + i, :].reshape(nh * NH, D)
                    c0 = i * NT8 * 128 + (base + p) * 128
                    whp8[:, c0:c0 + nh * NH] = blkw.T
        else:
            for dt in range(ndt):
                blkw = wu[:, :, dt, :].reshape(nh * NH, D)
                c0 = (base + dt) * 128
                whp16[:, c0:c0 + nh * NH] = blkw.T
        # mask row over t (broadcast over b at use site)
        for m2 in range(nh * NH):
            i = i0 + m2 // NH
            vt = min(L - i, nt)
            mrow[m2, u * 64 + vt:u * 64 + nt, 0] = NEG
        fcwh[:nh * NH, u * D:(u + 1) * D] = fcw_h[u * 128:u * 128 + nh * NH]
        bh_p[:nh * NH, u] = SC2 * bh[i0:i0 + nh].reshape(nh * NH)

    fcv = fc_W[:NV * D].reshape(NV, D, D)
    g = np.einsum("vt,vdk->tdk", Wv, fcv)            # [L, D, D] true units
    if G_FP8:
        gq = (g * SCALE).astype(f8).astype(f)
        g8 = np.zeros((D, 2 * NG * 128), f)
        for p in range(NG):
            for i in range(2):
                c0 = i * NG * 128 + p * 128
                g8[:, c0:c0 + 128] = gq[2 * p + i]
        g8 = g8.astype(f8)
    else:
        g8 = np.ascontiguousarray(
            (g * SCALE).transpose(1, 0, 2).reshape(D, L * D)).astype(b16)
    fcb = SC2 * (fc_b + np.einsum("v,vdk->k", bv, fcv)).reshape(1, D)

    return dict(item_emb=item_emb, user_emb=user_emb,
                whp8=whp8.astype(f8), whp16=whp16.astype(b16),
                g8=g8, fcwh=fcwh.astype(b16),
                mrow=mrow, bh_p=bh_p, fcb=fcb.astype(b16))


def make_in_maps(user_ids, item_seq, user_emb, item_emb, Wv, bv, Wh, bh,
                 fc_W, fc_b):
    common = _prep_common(user_emb, item_emb, Wv, bv, Wh, bh, fc_W, fc_b)
    user_ids = np.asarray(user_ids).astype(np.int32).reshape(B, 1)
    item_seq = np.asarray(item_seq).astype(np.int32).reshape(B, L)
    in_maps = []
    for c in range(NCORES):
        m = dict(common)
        m["seq_idx"] = np.ascontiguousarray(item_seq[c * BLOC:(c + 1) * BLOC])
        m["uid_idx"] = np.ascontiguousarray(user_ids[c * BLOC:(c + 1) * BLOC])
        in_maps.append(m)
    return in_maps


def get_nc():
    global _NC_CACHE
    if _NC_CACHE is None:
        _NC_CACHE = _build_nc()
        _install_wait_splitter(_NC_CACHE)
    return _NC_CACHE


def kernel(**inputs) -> np.ndarray:
    global LAST_RESULTS
    in_maps = make_in_maps(**inputs)
    nc = get_nc()
    trace = bool(int(os.environ.get("BASS_KERNEL_TRACE", "0")))
    res = run_bass_kernel_spmd(nc, in_maps, list(range(NCORES)), trace=trace)
    LAST_RESULTS = res
    return np.concatenate([res.results[c]["out"] for c in range(NCORES)], axis=0)
